# revision 3
# baseline (speedup 1.0000x reference)
"""Trainium2 Bass kernel v2 for nn_Decoder (12-step LSTM cell + BN/Linear
head), data-parallel over batch across 8 NeuronCores.

Key differences vs v1 baseline:
  * all big matmuls in bf16 (1 PE cycle/row vs 4 for fp32)
  * per-shard BN statistics (sharding hint allows it; stat error ~1e-3,
    tolerance 2e-2) -> no collectives at all
  * elementwise chain in bf16: DVE runs its TensorTensor/TensorScalar ops
    in 2x/4x perf modes
  * xp (the pure H @ W1.T partial) kept in SBUF in bf16, packed 2 steps
    deep: [100, 24576] (rows 0-49 even steps, 50-99 odd steps) -> phase-2
    passes have half the per-partition free size, and W2 is applied as a
    100->4 block-diagonal matmul
  * BN stats subsampled 1/4 (bn_stats hw max 512 free anyway)
  * relu folded into a DVE tensor_scalar: y' = max(A*xp, -C); the +C shift
    is pushed into the final output bias analytically
  * engine balance: ACT = 5 transcendental passes/step (bottleneck);
    DVE = t2/cn/hn muls + stats; Pool = t1 mul + xp PSUM->SBUF copy

Math recap (see v1 docstring): Wc = W_ih + W_hh, bc = b_ih + b_hh,
12 steps of z = Wc@h + bc; c' = sig(f)c + sig(i)tanh(g); h' = sig(o)tanh(c').
BN1/BN2/BN3 are whole-tensor-stat batchnorms -> scalar affine transforms,
derived per shard from (subsampled) channel stats.
"""

import sys

sys.path.insert(0, "/opt/trn_rl_repo")

import numpy as np
import ml_dtypes

import concourse.bass as bass
import concourse.mybir as mybir
import concourse.tile as tile
from concourse import bacc
from concourse.bass_utils import run_bass_kernel_spmd

AF = mybir.ActivationFunctionType
OP = mybir.AluOpType
FP32 = mybir.dt.float32
BF16 = mybir.dt.bfloat16

B = 32768
HID = 128
T = 12
NCORES = 8
BL = B // NCORES            # 4096 batch per core
QW = 1024                   # cols per quarter
NQ = BL // QW               # 4 quarters
XC = (T // 2) * BL          # 24576 cols of the packed xp buffer
XR = 114                    # xp partition rows: 0-49 even t, 64-113 odd t
                            # (engine partition base must be 0/32/64/96)
EPS = 1e-5
LAST_EXEC_NS = None

# gate order in the PyTorch weight layout: i, f, g, o
GI, GF, GG, GO = 0, 1, 2, 3


def build_nc(dbg=""):
    nc = bacc.Bacc(None, target_bir_lowering=False, debug=False)

    # ---------------- I/O ----------------
    hT = nc.dram_tensor("hT", [HID, BL], BF16, kind="ExternalInput")
    cT = nc.dram_tensor("cT", [HID, BL], BF16, kind="ExternalInput")
    WcT = nc.dram_tensor("WcT", [HID, 4 * HID], BF16, kind="ExternalInput")
    bcT = nc.dram_tensor("bcT", [HID, 4], FP32, kind="ExternalInput")
    W1T = nc.dram_tensor("W1T", [HID, 50], BF16, kind="ExternalInput")
    W2B = nc.dram_tensor("W2B", [XR, 4], BF16, kind="ExternalInput")
    W2Bf = nc.dram_tensor("W2Bf", [XR, 4], FP32, kind="ExternalInput")
    s1b = nc.dram_tensor("s1b", [XR, 1], FP32, kind="ExternalInput")
    b1b = nc.dram_tensor("b1b", [XR, 1], FP32, kind="ExternalInput")
    s2b = nc.dram_tensor("s2b", [4, 1], FP32, kind="ExternalInput")
    b2b = nc.dram_tensor("b2b", [4, 1], FP32, kind="ExternalInput")
    consts = nc.dram_tensor("consts", [1, 8], FP32, kind="ExternalInput")
    out_d = nc.dram_tensor("out", [4, XC], FP32, kind="ExternalOutput")
    if "x" in dbg:
        dbg_xp = nc.dram_tensor("dbg_xp", [XR, 1024], FP32,
                                kind="ExternalOutput")
    if "h" in dbg:
        dbg_h1 = nc.dram_tensor("dbg_h1", [HID, 512], FP32,
                                kind="ExternalOutput")
        dbg_h12 = nc.dram_tensor("dbg_h12", [HID, 512], FP32,
                                 kind="ExternalOutput")
    if "q" in dbg:
        dbg_scal = nc.dram_tensor("dbg_scal", [1, 16], FP32,
                                  kind="ExternalOutput")
    if "v" in dbg:
        dbg_vec = nc.dram_tensor("dbg_vec", [XR, 4], FP32,
                                 kind="ExternalOutput")

    with tile.TileContext(nc) as tc:
        import contextlib
        ctx = contextlib.ExitStack()
        with ctx:
            singles = ctx.enter_context(tc.tile_pool(name="singles", bufs=1))
            hpool = ctx.enter_context(tc.tile_pool(name="h", bufs=2))
            cpool = ctx.enter_context(tc.tile_pool(name="c", bufs=2))
            gt = ctx.enter_context(tc.tile_pool(name="gates", bufs=2))
            scal = ctx.enter_context(tc.tile_pool(name="scal", bufs=1))
            spool = ctx.enter_context(tc.tile_pool(name="stage", bufs=2))
            psum_ctx = contextlib.ExitStack()
            gpa = psum_ctx.enter_context(
                tc.tile_pool(name="gpa", bufs=1, space="PSUM"))
            gpb = psum_ctx.enter_context(
                tc.tile_pool(name="gpb", bufs=1, space="PSUM"))
            xf = psum_ctx.enter_context(
                tc.tile_pool(name="xf", bufs=2, space="PSUM"))

            # ---------------- loads (critical path first) ----------------
            wct = singles.tile([HID, 4 * HID], BF16)
            nc.sync.dma_start(out=wct[:], in_=WcT[:, :])
            bct = singles.tile([HID, 4], FP32)
            nc.sync.dma_start(out=bct[:], in_=bcT[:, :])
            h0 = hpool.tile([HID, BL], BF16)
            c0 = cpool.tile([HID, BL], BF16)
            for k in range(8):
                s = slice(k * 512, (k + 1) * 512)
                nc.sync.dma_start(out=h0[:, s], in_=hT[:, s])
                nc.sync.dma_start(out=c0[:, s], in_=cT[:, s])
            w1t = singles.tile([HID, 50], BF16)
            nc.sync.dma_start(out=w1t[:], in_=W1T[:, :])
            w2t = singles.tile([XR, 4], BF16)
            nc.sync.dma_start(out=w2t[:], in_=W2B[:, :])
            w2tf = singles.tile([XR, 4], FP32)
            nc.sync.dma_start(out=w2tf[:], in_=W2Bf[:, :])
            s1t = singles.tile([XR, 1], FP32)
            nc.sync.dma_start(out=s1t[:], in_=s1b[:, :])
            b1t = singles.tile([XR, 1], FP32)
            nc.sync.dma_start(out=b1t[:], in_=b1b[:, :])
            s2t = singles.tile([4, 1], FP32)
            nc.sync.dma_start(out=s2t[:], in_=s2b[:, :])
            b2t = singles.tile([4, 1], FP32)
            nc.sync.dma_start(out=b2t[:], in_=b2b[:, :])
            cst = singles.tile([1, 8], FP32)
            nc.sync.dma_start(out=cst[:], in_=consts[:, :])
            ones = singles.tile([HID, 1], FP32)
            nc.vector.memset(ones[:], 1.0)
            ones_row = singles.tile([1, HID], FP32)
            nc.vector.memset(ones_row[:], 1.0)

            xp_all = singles.tile([XR, XC], BF16)
            statsH = singles.tile([HID, 2 * T, 6], FP32)
            statsX = singles.tile([XR, T, 6], FP32)
            statsY = singles.tile([XR, T // 2, 6], FP32)
            # dead partition rows 50-63 are never written by the packed
            # stores but ARE covered by full-[XR] reads -> zero them.
            # The xp memset goes on Pool in per-quarter chunks (see the
            # step loop) so it doesn't head-of-line-block Pool's t1 muls.
            nc.vector.memset(statsX[32:64, :, :], 0.0)
            nc.vector.memset(statsY[32:64, :, :], 0.0)
            ms_chunks = [(k * 2048, 2048) for k in range(XC // 2048)]

            # ---------------- LSTM ----------------
            # Deferred "flush" (tanh(c')/h'/xp) runs 2 quarters behind the
            # gate pipeline and crosses step boundaries, so ACT never stalls
            # at the end of a step waiting on the Pool->DVE c' chain.
            pending = []

            def flush_one():
                ft, fq, fso, fhn, fcn = pending.pop(0)
                q0 = fq * QW
                tcn = gt.tile([HID, QW], BF16, tag="tcn")
                nc.scalar.activation(tcn[:], fcn[:, q0:q0 + QW], AF.Tanh)
                nc.vector.tensor_mul(fhn[:, q0:q0 + QW], fso[:], tcn[:])
                xq = xf.tile([50, QW], FP32, tag="xf")
                nc.tensor.matmul(xq[:, 0:512], w1t[:],
                                 fhn[:, q0:q0 + 512], start=True, stop=True)
                nc.tensor.matmul(xq[:, 512:QW], w1t[:],
                                 fhn[:, q0 + 512:q0 + QW],
                                 start=True, stop=True)
                r0 = (ft % 2) * 64
                cbase = (ft // 2) * BL + q0
                dst = xp_all[r0:r0 + 50, cbase:cbase + QW]
                # Pool cannot read PSUM on TRN2 -> DVE does this copy
                nc.vector.tensor_copy(dst, xq[:])
                if fq in (0, 2):
                    e = 2 * ft + fq // 2
                    nc.vector.bn_stats(out=statsH[:, e, :],
                                       in_=fhn[:, q0:q0 + 512])
                    ex = 2 * (ft // 2) + fq // 2
                    nc.vector.bn_stats(
                        out=statsX[r0:r0 + 50, ex, :],
                        in_=xp_all[r0:r0 + 50, cbase:cbase + 512])

            hc, cc = h0, c0
            for t in range(T):
                hn = hpool.tile([HID, BL], BF16)
                cn = cpool.tile([HID, BL], BF16)
                for q in range(NQ):
                    q0 = q * QW
                    ga = gpa.tile([HID, QW], FP32, tag="ga")
                    nc.tensor.matmul(ga[:, 0:512], wct[:, GI * HID:(GI + 1) * HID],
                                     hc[:, q0:q0 + 512], start=True, stop=True)
                    nc.tensor.matmul(ga[:, 512:QW], wct[:, GI * HID:(GI + 1) * HID],
                                     hc[:, q0 + 512:q0 + QW], start=True, stop=True)
                    gb = gpb.tile([HID, QW], FP32, tag="gb")
                    nc.tensor.matmul(gb[:, 0:512], wct[:, GG * HID:(GG + 1) * HID],
                                     hc[:, q0:q0 + 512], start=True, stop=True)
                    nc.tensor.matmul(gb[:, 512:QW], wct[:, GG * HID:(GG + 1) * HID],
                                     hc[:, q0 + 512:q0 + QW], start=True, stop=True)
                    si = gt.tile([HID, QW], BF16, tag="si")
                    nc.scalar.activation(si[:], ga[:], AF.Sigmoid,
                                         bias=bct[:, GI:GI + 1])
                    tg = gt.tile([HID, QW], BF16, tag="tg")
                    nc.scalar.activation(tg[:], gb[:], AF.Tanh,
                                         bias=bct[:, GG:GG + 1])
                    ga2 = gpa.tile([HID, QW], FP32, tag="ga")
                    nc.tensor.matmul(ga2[:, 0:512], wct[:, GF * HID:(GF + 1) * HID],
                                     hc[:, q0:q0 + 512], start=True, stop=True)
                    nc.tensor.matmul(ga2[:, 512:QW], wct[:, GF * HID:(GF + 1) * HID],
                                     hc[:, q0 + 512:q0 + QW], start=True, stop=True)
                    gb2 = gpb.tile([HID, QW], FP32, tag="gb")
                    nc.tensor.matmul(gb2[:, 0:512], wct[:, GO * HID:(GO + 1) * HID],
                                     hc[:, q0:q0 + 512], start=True, stop=True)
                    nc.tensor.matmul(gb2[:, 512:QW], wct[:, GO * HID:(GO + 1) * HID],
                                     hc[:, q0 + 512:q0 + QW], start=True, stop=True)
                    # flush the (q-2) tail HERE so its DVE/PE ops land ahead
                    # of this quarter's t2/cn in the in-order engine queues
                    # (otherwise next quarter's gate matmul transitively
                    # waits on the Pool t1 mul)
                    if len(pending) > 1:
                        flush_one()
                    sf = gt.tile([HID, QW], BF16, tag="sf")
                    nc.scalar.activation(sf[:], ga2[:], AF.Sigmoid,
                                         bias=bct[:, GF:GF + 1])
                    so = gt.tile([HID, QW], BF16, tag="so", bufs=3)
                    nc.scalar.activation(so[:], gb2[:], AF.Sigmoid,
                                         bias=bct[:, GO:GO + 1])
                    # t2 = sig(i)*tanh(g)  (in place on tg, DVE bf16 2x)
                    nc.vector.tensor_mul(tg[:], si[:], tg[:])
                    # t1 = sig(f)*c        (in place on sf, Pool)
                    nc.gpsimd.tensor_mul(sf[:], sf[:], cc[:, q0:q0 + QW])
                    if ms_chunks:
                        mo, mw = ms_chunks.pop(0)
                        nc.gpsimd.memset(xp_all[32:64, mo:mo + mw], 0.0)
                    # c_new = t1 + t2      (DVE bf16 2x)
                    nc.vector.tensor_add(cn[:, q0:q0 + QW], sf[:], tg[:])
                    pending.append((t, q, so, hn, cn))
                hc, cc = hn, cn
            while pending:
                flush_one()
            if "h" in dbg:
                dh = scal.tile([HID, 512], FP32)
                nc.vector.tensor_copy(dh[:], hc[:, 0:512])
                nc.sync.dma_start(out=dbg_h12[:, :], in_=dh[:])

            # ---------------- stats finalize #1 (per-shard) -------------
            def mk_chain():
                ctr = [0]
                def mk():
                    ctr[0] += 1
                    return scal.tile([1, 1], FP32, name=f"sc{ctr[0]}",
                                     tag=f"sc{ctr[0]}")
                return mk
            mk = mk_chain()
            eps_t = scal.tile([1, 1], FP32)
            nc.vector.memset(eps_t[:], EPS)
            c15 = scal.tile([1, 1], FP32)
            nc.vector.memset(c15[:], 1.5)

            def rstd_of(v):
                """1/sqrt(v+eps); ACT sqrt + exact DVE reciprocal is ~1e-3
                accurate which is plenty for the 2e-2 gate"""
                rt = mk()
                nc.scalar.activation(rt[:], v[:], AF.Sqrt, bias=eps_t[0:1])
                r = mk(); nc.vector.reciprocal(r[:], rt[:])
                return r

            # channel-wise mean / E[x^2] of H and X
            mvH = scal.tile([HID, 2], FP32)
            nc.vector.bn_aggr(out=mvH[:], in_=statsH[:].rearrange(
                "p a b -> p (a b)"))
            e2H = scal.tile([HID, 1], FP32)
            nc.vector.tensor_mul(e2H[:], mvH[:, 0:1], mvH[:, 0:1])
            nc.vector.tensor_add(e2H[:], e2H[:], mvH[:, 1:2])
            mvX = scal.tile([XR, 2], FP32)
            nc.vector.bn_aggr(out=mvX[:], in_=statsX[:].rearrange(
                "p a b -> p (a b)"))
            e2X = scal.tile([XR, 1], FP32)
            nc.vector.tensor_mul(e2X[:], mvX[:, 0:1], mvX[:, 0:1])
            nc.vector.tensor_add(e2X[:], e2X[:], mvX[:, 1:2])

            # global BN1 stats via ones-matmul
            smat = scal.tile([HID, 2], FP32)
            nc.vector.tensor_copy(smat[:, 0:1], mvH[:, 0:1])
            nc.vector.tensor_copy(smat[:, 1:2], e2H[:])
            sp1 = gpa.tile([HID, QW], FP32, tag="ga")
            nc.tensor.matmul(sp1[0:1, 0:2], ones[:], smat[:],
                             start=True, stop=True)
            srow = scal.tile([1, 2], FP32)
            nc.vector.tensor_copy(srow[:], sp1[0:1, 0:2])

            m1 = mk(); nc.scalar.mul(m1[:], srow[:, 0:1], 1.0 / HID)
            E2 = mk(); nc.scalar.mul(E2[:], srow[:, 1:2], 1.0 / HID)
            msq = mk(); nc.vector.tensor_mul(msq[:], m1[:], m1[:])
            v1 = mk(); nc.vector.tensor_sub(v1[:], E2[:], msq[:])
            rstd1 = rstd_of(v1)
            a1 = mk(); nc.vector.tensor_mul(a1[:], rstd1[:], cst[:, 0:1])
            bb = mk(); nc.vector.tensor_mul(bb[:], m1[:], a1[:])
            nc.vector.tensor_sub(bb[:], cst[:, 1:2], bb[:])

            # broadcast bb to 100 partitions via PE; c1 = bb*s1 + b1
            bc_ps = gpa.tile([HID, QW], FP32, tag="ga")
            nc.tensor.matmul(bc_ps[0:XR, 0:1], ones_row[:, 0:XR], bb[:],
                             start=True, stop=True)
            bb_b = scal.tile([XR, 1], FP32)
            nc.vector.tensor_copy(bb_b[:], bc_ps[0:XR, 0:1])
            c1 = scal.tile([XR, 1], FP32)
            nc.vector.tensor_scalar(out=c1[:], in0=s1t[:], scalar1=bb_b[:],
                                    scalar2=b1t[:], op0=OP.mult, op1=OP.add)

            # global BN2 stats: x = a1*xp + c1
            smat2 = scal.tile([HID, 5], FP32)
            nc.vector.memset(smat2[:], 0.0)
            for lo, hi in ((0, 50), (64, XR)):
                s_ = slice(lo, hi)
                nc.vector.tensor_copy(smat2[s_, 0:1], mvX[s_, 0:1])
                nc.vector.tensor_copy(smat2[s_, 1:2], e2X[s_, :])
                nc.vector.tensor_copy(smat2[s_, 2:3], c1[s_, :])
                nc.vector.tensor_mul(smat2[s_, 3:4], c1[s_, :], mvX[s_, 0:1])
                nc.vector.tensor_mul(smat2[s_, 4:5], c1[s_, :], c1[s_, :])
            sp2 = gpa.tile([HID, QW], FP32, tag="ga")
            nc.tensor.matmul(sp2[0:1, 0:5], ones[:], smat2[:],
                             start=True, stop=True)
            srow2 = scal.tile([1, 5], FP32)
            nc.vector.tensor_copy(srow2[:], sp2[0:1, 0:5])

            # m2 = a1*mean(mX) + mean(c1)
            t1_ = mk(); nc.vector.tensor_mul(t1_[:], a1[:], srow2[:, 0:1])
            m2 = mk()
            nc.vector.tensor_add(m2[:], t1_[:], srow2[:, 2:3])
            nc.scalar.mul(m2[:], m2[:], 1.0 / 100.0)
            # E[x^2] = a1^2*mean(e2X) + 2*a1*mean(c1*mX) + mean(c1^2)
            a1sq = mk(); nc.vector.tensor_mul(a1sq[:], a1[:], a1[:])
            u1 = mk(); nc.vector.tensor_mul(u1[:], a1sq[:], srow2[:, 1:2])
            u2 = mk(); nc.vector.tensor_mul(u2[:], a1[:], srow2[:, 3:4])
            nc.scalar.mul(u2[:], u2[:], 2.0)
            nc.vector.tensor_add(u1[:], u1[:], u2[:])
            nc.vector.tensor_add(u1[:], u1[:], srow2[:, 4:5])
            E2x = mk(); nc.scalar.mul(E2x[:], u1[:], 1.0 / 100.0)
            m2sq = mk(); nc.vector.tensor_mul(m2sq[:], m2[:], m2[:])
            v2 = mk(); nc.vector.tensor_sub(v2[:], E2x[:], m2sq[:])
            rstd2 = rstd_of(v2)
            a2 = mk(); nc.vector.tensor_mul(a2[:], rstd2[:], cst[:, 2:3])
            b2a = mk(); nc.vector.tensor_mul(b2a[:], m2[:], a2[:])
            nc.vector.tensor_sub(b2a[:], cst[:, 3:4], b2a[:])
            A = mk(); nc.vector.tensor_mul(A[:], a2[:], a1[:])

            # broadcast (A, a2, b2a) to 100 partitions
            pk = scal.tile([1, 3], FP32)
            nc.vector.tensor_copy(pk[:, 0:1], A[:])
            nc.vector.tensor_copy(pk[:, 1:2], a2[:])
            nc.vector.tensor_copy(pk[:, 2:3], b2a[:])
            bc2 = gpa.tile([HID, QW], FP32, tag="ga")
            nc.tensor.matmul(bc2[0:XR, 0:3], ones_row[:, 0:XR], pk[:],
                             start=True, stop=True)
            bcs = scal.tile([XR, 3], FP32)
            nc.vector.tensor_copy(bcs[:], bc2[0:XR, 0:3])
            A_b = bcs[:, 0:1]
            Cv = scal.tile([XR, 1], FP32)        # C = a2*c1 + b2a
            nc.vector.tensor_scalar(out=Cv[:], in0=c1[:], scalar1=bcs[:, 1:2],
                                    scalar2=bcs[:, 2:3], op0=OP.mult,
                                    op1=OP.add)
            negC = scal.tile([XR, 1], FP32)
            nc.scalar.mul(negC[:], Cv[:], -1.0)

            if "q" in dbg:
                dsc = scal.tile([1, 16], FP32)
                nc.vector.memset(dsc[:], 0.0)
                for k_, v_ in enumerate([m1, v1, rstd1, a1, bb, m2, v2,
                                         rstd2, a2, b2a, A]):
                    nc.vector.tensor_copy(dsc[:, k_:k_ + 1], v_[:])
                nc.sync.dma_start(out=dbg_scal[:, :], in_=dsc[:])
            if "x" in dbg:
                dx = scal.tile([XR, 1024], FP32)
                nc.vector.tensor_copy(dx[:], xp_all[:, 0:1024])
                nc.sync.dma_start(out=dbg_xp[:, :], in_=dx[:])

            # ---------------- pass 2a: y' = max(A*xp, -C), stats ---------
            NCH = XC // 2048                     # 12 chunks
            for ch in range(NCH):
                cs = ch * 2048
                sl = xp_all[:, cs:cs + 2048]
                nc.vector.tensor_scalar(out=sl, in0=sl, scalar1=A_b,
                                        scalar2=negC[:], op0=OP.mult,
                                        op1=OP.max)
                if ch % 2 == 0:
                    nc.vector.bn_stats(out=statsY[:, ch // 2, :],
                                       in_=xp_all[:, cs:cs + 512])

            # ---------------- stats finalize #2 (BN3) --------------------
            mvY = scal.tile([XR, 2], FP32)
            nc.vector.bn_aggr(out=mvY[:], in_=statsY[:].rearrange(
                "p a b -> p (a b)"))
            e2Y = scal.tile([XR, 1], FP32)
            nc.vector.tensor_mul(e2Y[:], mvY[:, 0:1], mvY[:, 0:1])
            nc.vector.tensor_add(e2Y[:], e2Y[:], mvY[:, 1:2])
            # y = y' + C: mean/E2 shift
            smat3 = scal.tile([HID, 5], FP32)
            nc.vector.memset(smat3[:], 0.0)
            for lo, hi in ((0, 50), (64, XR)):
                s_ = slice(lo, hi)
                nc.vector.tensor_copy(smat3[s_, 0:1], mvY[s_, 0:1])
                nc.vector.tensor_copy(smat3[s_, 1:2], e2Y[s_, :])
                nc.vector.tensor_copy(smat3[s_, 2:3], Cv[s_, :])
                nc.vector.tensor_mul(smat3[s_, 3:4], Cv[s_, :], mvY[s_, 0:1])
                nc.vector.tensor_mul(smat3[s_, 4:5], Cv[s_, :], Cv[s_, :])
            sp3 = gpa.tile([HID, QW], FP32, tag="ga")
            nc.tensor.matmul(sp3[0:1, 0:5], ones[:], smat3[:],
                             start=True, stop=True)
            srow3 = scal.tile([1, 5], FP32)
            nc.vector.tensor_copy(srow3[:], sp3[0:1, 0:5])

            m3 = mk()
            nc.vector.tensor_add(m3[:], srow3[:, 0:1], srow3[:, 2:3])
            nc.scalar.mul(m3[:], m3[:], 1.0 / 100.0)
            w1_ = mk(); nc.scalar.mul(w1_[:], srow3[:, 3:4], 2.0)
            nc.vector.tensor_add(w1_[:], w1_[:], srow3[:, 1:2])
            nc.vector.tensor_add(w1_[:], w1_[:], srow3[:, 4:5])
            E3 = mk(); nc.scalar.mul(E3[:], w1_[:], 1.0 / 100.0)
            m3sq = mk(); nc.vector.tensor_mul(m3sq[:], m3[:], m3[:])
            v3 = mk(); nc.vector.tensor_sub(v3[:], E3[:], m3sq[:])
            rstd3 = rstd_of(v3)
            a3 = mk(); nc.vector.tensor_mul(a3[:], rstd3[:], cst[:, 4:5])
            b3a = mk(); nc.vector.tensor_mul(b3a[:], m3[:], a3[:])
            nc.vector.tensor_sub(b3a[:], cst[:, 5:6], b3a[:])

            # CW2[j] = sum_p C_p * W2blk[p, j]
            cw_ps = gpa.tile([HID, QW], FP32, tag="ga")
            nc.tensor.matmul(cw_ps[0:4, 0:1], w2tf[:], Cv[:],
                             start=True, stop=True)
            CW2 = scal.tile([4, 1], FP32)
            nc.vector.tensor_copy(CW2[:], cw_ps[0:4, 0:1])
            # broadcast a3, b3a to 4 partitions
            pk3 = scal.tile([1, 2], FP32)
            nc.vector.tensor_copy(pk3[:, 0:1], a3[:])
            nc.vector.tensor_copy(pk3[:, 1:2], b3a[:])
            bc3 = gpa.tile([HID, QW], FP32, tag="ga")
            nc.tensor.matmul(bc3[0:4, 0:2], ones_row[:, 0:4], pk3[:],
                             start=True, stop=True)
            ab3 = scal.tile([4, 2], FP32)
            nc.vector.tensor_copy(ab3[:], bc3[0:4, 0:2])
            a3_b = ab3[:, 0:1]
            # cbv = a3*CW2 + b3a*s2blk + b2blk
            cbv = scal.tile([4, 1], FP32)
            nc.vector.tensor_scalar(out=cbv[:], in0=s2t[:],
                                    scalar1=ab3[:, 1:2], scalar2=b2t[:],
                                    op0=OP.mult, op1=OP.add)
            t4 = scal.tile([4, 1], FP32)
            nc.vector.tensor_mul(t4[:], CW2[:], ab3[:, 0:1])
            nc.vector.tensor_add(cbv[:], cbv[:], t4[:])

            if "v" in dbg:
                dvc = scal.tile([XR, 4], FP32)
                nc.vector.tensor_copy(dvc[:, 0:1], c1[:])
                nc.vector.tensor_copy(dvc[:, 1:2], Cv[:])
                nc.vector.tensor_copy(dvc[:, 2:3], mvX[:, 0:1])
                nc.vector.tensor_copy(dvc[:, 3:4], e2X[:])
                nc.sync.dma_start(out=dbg_vec[:, :], in_=dvc[:])

            # ---------------- pass 2b: out = a3*(y' @ W2blk) + cbv -------
            # final affine+copy split across ACT/DVE/Pool so the tail is
            # paced by three engines instead of one
            psum_ctx.close()
            rp = ctx.enter_context(
                tc.tile_pool(name="rp", bufs=4, space="PSUM"))
            # engine rotation (Pool cannot read PSUM): ACT 7 : DVE 5
            ENG = [0, 1, 0, 1, 0, 1, 0, 0, 1, 0, 1, 0]
            # stage 4096-wide, DMA once per 4 affine chunks (HWDGE desc-gen
            # is ~625ns per dma_start — 24 small DMAs would pace the tail)
            stg = None
            for ch in range(XC // 1024):
                cs = ch * 1024
                r2_ = rp.tile([4, 1024], FP32, tag="rp")
                for j in range(2):
                    csl = slice(cs + j * 512, cs + (j + 1) * 512)
                    nc.tensor.matmul(
                        r2_[:, j * 512:(j + 1) * 512], w2t[:],
                        xp_all[:, csl], start=True, stop=True)
                if ch % 4 == 0:
                    stg = spool.tile([4, 4096], FP32, tag="os", bufs=2)
                so_ = stg[:, (ch % 4) * 1024:(ch % 4 + 1) * 1024]
                eng = ENG[ch % 12]
                if eng == 0:
                    nc.scalar.activation(so_, r2_[:], AF.Identity,
                                         bias=cbv[:], scale=a3_b)
                else:
                    nc.vector.tensor_scalar(out=so_, in0=r2_[:],
                                            scalar1=a3_b, scalar2=cbv[:],
                                            op0=OP.mult, op1=OP.add)
                if ch % 4 == 3:
                    nc.sync.dma_start(out=out_d[:, cs - 3072:cs + 1024],
                                      in_=stg[:])

    nc.finalize()
    return nc


_NC_CACHE = {}


def _get_nc(dbg=""):
    if dbg not in _NC_CACHE:
        _NC_CACHE[dbg] = build_nc(dbg)
    return _NC_CACHE[dbg]


def kernel(h, c, W_ih, W_hh, b_ih, b_hh, gamma1, beta1, gamma2, beta2,
           gamma3, beta3, W1, b1, W2, b2, dbg=""):
    h = np.asarray(h, np.float32)
    c = np.asarray(c, np.float32)
    W_ih = np.asarray(W_ih, np.float32)
    W_hh = np.asarray(W_hh, np.float32)
    b_ih = np.asarray(b_ih, np.float32)
    b_hh = np.asarray(b_hh, np.float32)
    W1 = np.asarray(W1, np.float32)
    b1 = np.asarray(b1, np.float32)
    W2 = np.asarray(W2, np.float32)
    b2 = np.asarray(b2, np.float32)
    bf = ml_dtypes.bfloat16

    hT = np.ascontiguousarray(h[0].T.astype(bf))          # [128, B] bf16
    cT = np.ascontiguousarray(c[0].T.astype(bf))
    Wc = W_ih + W_hh                                      # [512, 128]
    WcT = np.ascontiguousarray(Wc.T.astype(bf))           # [128, 512]
    bc = b_ih + b_hh
    bcT = np.ascontiguousarray(bc.reshape(4, HID).T)      # [128, 4] fp32
    W1T = np.ascontiguousarray(W1.T.astype(bf))           # [128, 50]
    # block-diagonal W2^T over 2 timesteps: [100, 4]
    W2B = np.zeros((114, 4), np.float32)
    W2B[0:50, 0:2] = W2.T
    W2B[64:114, 2:4] = W2.T
    s1 = W1.sum(1)                                        # [50]
    s1b = np.zeros((114, 1), np.float32)
    s1b[0:50, 0], s1b[64:114, 0] = s1, s1
    b1b = np.zeros((114, 1), np.float32)
    b1b[0:50, 0], b1b[64:114, 0] = b1, b1
    s2 = W2.sum(1)                                        # [2]
    s2b = np.ascontiguousarray(np.tile(s2, 2)[:, None])   # [4,1]
    b2b = np.ascontiguousarray(np.tile(b2, 2)[:, None])
    consts = np.array([[float(gamma1), float(beta1), float(gamma2),
                        float(beta2), float(gamma3), float(beta3), 0.0, 0.0]],
                      np.float32)

    shared = {"WcT": WcT, "bcT": bcT, "W1T": W1T,
              "W2B": W2B.astype(bf), "W2Bf": W2B,
              "s1b": s1b, "b1b": b1b, "s2b": s2b, "b2b": b2b,
              "consts": consts}
    in_maps = []
    for i in range(NCORES):
        s = slice(i * BL, (i + 1) * BL)
        in_maps.append({"hT": np.ascontiguousarray(hT[:, s]),
                        "cT": np.ascontiguousarray(cT[:, s]), **shared})

    nc = _get_nc(dbg)
    res = run_bass_kernel_spmd(nc, in_maps, list(range(NCORES)))
    global LAST_EXEC_NS
    if getattr(res, "exec_time_ns", None):
        LAST_EXEC_NS = res.exec_time_ns
    if dbg:
        return res

    out = np.empty((B, T, 2), np.float32)
    for i in range(NCORES):
        arr = res.results[i]["out"]              # [4, 24576]
        # row = (t%2)*2 + ch ; col = (t//2)*4096 + b
        a4 = arr.reshape(2, 2, T // 2, BL)       # [parity, ch, pair, b]
        out[i * BL:(i + 1) * BL] = a4.transpose(3, 2, 0, 1).reshape(BL, T, 2)
    return out


# revision 4
# speedup vs baseline: 1.0149x; 1.0149x over previous
"""Trainium2 Bass kernel v2 for nn_Decoder (12-step LSTM cell + BN/Linear
head), data-parallel over batch across 8 NeuronCores.

Key differences vs v1 baseline:
  * all big matmuls in bf16 (1 PE cycle/row vs 4 for fp32)
  * per-shard BN statistics (sharding hint allows it; stat error ~1e-3,
    tolerance 2e-2) -> no collectives at all
  * elementwise chain in bf16: DVE runs its TensorTensor/TensorScalar ops
    in 2x/4x perf modes
  * xp (the pure H @ W1.T partial) kept in SBUF in bf16, packed 2 steps
    deep: [114, 24576] (rows 0-49 even steps, 64-113 odd steps; engine
    partition bases must be 0/32/64/96) -> phase-2 passes have half the
    per-partition free size, and W2 is applied as a block-diagonal matmul
  * BN stats subsampled 1/4 (bn_stats hw max 512 free anyway)
  * relu folded into a DVE tensor_scalar: y' = max(A*xp, -C); the +C shift
    is pushed into the final output bias analytically
  * engine balance: ACT = 5 transcendental passes/step (bottleneck);
    DVE = t2/cn/hn muls + stats; Pool = t1 mul + xp PSUM->SBUF copy

Math recap (see v1 docstring): Wc = W_ih + W_hh, bc = b_ih + b_hh,
12 steps of z = Wc@h + bc; c' = sig(f)c + sig(i)tanh(g); h' = sig(o)tanh(c').
BN1/BN2/BN3 are whole-tensor-stat batchnorms -> scalar affine transforms,
derived per shard from (subsampled) channel stats.
"""

import sys

sys.path.insert(0, "/opt/trn_rl_repo")

import numpy as np
import ml_dtypes

import concourse.bass as bass
import concourse.mybir as mybir
import concourse.tile as tile
from concourse import bacc
from concourse.bass_utils import run_bass_kernel_spmd

AF = mybir.ActivationFunctionType
OP = mybir.AluOpType
FP32 = mybir.dt.float32
BF16 = mybir.dt.bfloat16

B = 32768
HID = 128
T = 12
NCORES = 8
BL = B // NCORES            # 4096 batch per core
QW = 1024                   # cols per quarter
NQ = BL // QW               # 4 quarters
XC = (T // 2) * BL          # 24576 cols of the packed xp buffer
XR = 114                    # xp partition rows: 0-49 even t, 64-113 odd t
                            # (engine partition base must be 0/32/64/96)
EPS = 1e-5
LAST_EXEC_NS = None

# gate order in the PyTorch weight layout: i, f, g, o
GI, GF, GG, GO = 0, 1, 2, 3


def build_nc(dbg=""):
    nc = bacc.Bacc(None, target_bir_lowering=False, debug=False)

    # ---------------- I/O ----------------
    hT = nc.dram_tensor("hT", [HID, BL], BF16, kind="ExternalInput")
    cT = nc.dram_tensor("cT", [HID, BL], BF16, kind="ExternalInput")
    WcT = nc.dram_tensor("WcT", [HID, 4 * HID], BF16, kind="ExternalInput")
    bcT = nc.dram_tensor("bcT", [HID, 4], FP32, kind="ExternalInput")
    W1T = nc.dram_tensor("W1T", [HID, 50], BF16, kind="ExternalInput")
    W2B = nc.dram_tensor("W2B", [XR, 4], BF16, kind="ExternalInput")
    W2Bf = nc.dram_tensor("W2Bf", [XR, 4], FP32, kind="ExternalInput")
    s1b = nc.dram_tensor("s1b", [XR, 1], FP32, kind="ExternalInput")
    b1b = nc.dram_tensor("b1b", [XR, 1], FP32, kind="ExternalInput")
    s2b = nc.dram_tensor("s2b", [4, 1], FP32, kind="ExternalInput")
    b2b = nc.dram_tensor("b2b", [4, 1], FP32, kind="ExternalInput")
    consts = nc.dram_tensor("consts", [1, 8], FP32, kind="ExternalInput")
    out_d = nc.dram_tensor("out", [4, XC], FP32, kind="ExternalOutput")
    if "x" in dbg:
        dbg_xp = nc.dram_tensor("dbg_xp", [XR, 1024], FP32,
                                kind="ExternalOutput")
    if "h" in dbg:
        dbg_h1 = nc.dram_tensor("dbg_h1", [HID, 512], FP32,
                                kind="ExternalOutput")
        dbg_h12 = nc.dram_tensor("dbg_h12", [HID, 512], FP32,
                                 kind="ExternalOutput")
    if "q" in dbg:
        dbg_scal = nc.dram_tensor("dbg_scal", [1, 16], FP32,
                                  kind="ExternalOutput")
    if "v" in dbg:
        dbg_vec = nc.dram_tensor("dbg_vec", [XR, 4], FP32,
                                 kind="ExternalOutput")

    with tile.TileContext(nc) as tc:
        import contextlib
        ctx = contextlib.ExitStack()
        with ctx:
            singles = ctx.enter_context(tc.tile_pool(name="singles", bufs=1))
            hpool = ctx.enter_context(tc.tile_pool(name="h", bufs=2))
            cpool = ctx.enter_context(tc.tile_pool(name="c", bufs=2))
            gt = ctx.enter_context(tc.tile_pool(name="gates", bufs=2))
            scal = ctx.enter_context(tc.tile_pool(name="scal", bufs=1))
            spool = ctx.enter_context(tc.tile_pool(name="stage", bufs=2))
            psum_ctx = contextlib.ExitStack()
            gpa = psum_ctx.enter_context(
                tc.tile_pool(name="gpa", bufs=1, space="PSUM"))
            gpb = psum_ctx.enter_context(
                tc.tile_pool(name="gpb", bufs=1, space="PSUM"))
            xf = psum_ctx.enter_context(
                tc.tile_pool(name="xf", bufs=2, space="PSUM"))

            # ---------------- loads (critical path first) ----------------
            wct = singles.tile([HID, 4 * HID], BF16)
            nc.sync.dma_start(out=wct[:], in_=WcT[:, :])
            bct = singles.tile([HID, 4], FP32)
            nc.sync.dma_start(out=bct[:], in_=bcT[:, :])
            h0 = hpool.tile([HID, BL], BF16)
            c0 = cpool.tile([HID, BL], BF16)
            for k in range(8):
                s = slice(k * 512, (k + 1) * 512)
                nc.sync.dma_start(out=h0[:, s], in_=hT[:, s])
                nc.sync.dma_start(out=c0[:, s], in_=cT[:, s])
            w1t = singles.tile([HID, 50], BF16)
            nc.sync.dma_start(out=w1t[:], in_=W1T[:, :])
            w2t = singles.tile([XR, 4], BF16)
            nc.sync.dma_start(out=w2t[:], in_=W2B[:, :])
            w2tf = singles.tile([XR, 4], FP32)
            nc.sync.dma_start(out=w2tf[:], in_=W2Bf[:, :])
            s1t = singles.tile([XR, 1], FP32)
            nc.sync.dma_start(out=s1t[:], in_=s1b[:, :])
            b1t = singles.tile([XR, 1], FP32)
            nc.sync.dma_start(out=b1t[:], in_=b1b[:, :])
            s2t = singles.tile([4, 1], FP32)
            nc.sync.dma_start(out=s2t[:], in_=s2b[:, :])
            b2t = singles.tile([4, 1], FP32)
            nc.sync.dma_start(out=b2t[:], in_=b2b[:, :])
            cst = singles.tile([1, 8], FP32)
            nc.sync.dma_start(out=cst[:], in_=consts[:, :])
            ones = singles.tile([HID, 1], FP32)
            nc.vector.memset(ones[:], 1.0)
            ones_row = singles.tile([1, HID], FP32)
            nc.vector.memset(ones_row[:], 1.0)

            xp_all = singles.tile([XR, XC], BF16)
            statsH = singles.tile([HID, 2 * T, 6], FP32)
            statsX = singles.tile([XR, T, 6], FP32)
            statsY = singles.tile([XR, T // 2, 6], FP32)
            # dead partition rows 50-63 are never written by the packed
            # stores but ARE covered by full-[XR] reads -> zero them.
            # The xp memset goes on Pool in per-quarter chunks (see the
            # step loop) so it doesn't head-of-line-block Pool's t1 muls.
            nc.vector.memset(statsX[32:64, :, :], 0.0)
            nc.vector.memset(statsY[32:64, :, :], 0.0)
            ms_chunks = [(k * 2048, 2048) for k in range(XC // 2048)]

            # ---------------- LSTM ----------------
            # Deferred "flush" (tanh(c')/h'/xp) runs 2 quarters behind the
            # gate pipeline and crosses step boundaries, so ACT never stalls
            # at the end of a step waiting on the Pool->DVE c' chain.
            pending = []

            def flush_one():
                ft, fq, fso, fhn, fcn = pending.pop(0)
                q0 = fq * QW
                tcn = gt.tile([HID, QW], BF16, tag="tcn")
                nc.scalar.activation(tcn[:], fcn[:, q0:q0 + QW], AF.Tanh)
                nc.vector.tensor_mul(fhn[:, q0:q0 + QW], fso[:], tcn[:])
                xq = xf.tile([50, QW], FP32, tag="xf")
                nc.tensor.matmul(xq[:, 0:512], w1t[:],
                                 fhn[:, q0:q0 + 512], start=True, stop=True)
                nc.tensor.matmul(xq[:, 512:QW], w1t[:],
                                 fhn[:, q0 + 512:q0 + QW],
                                 start=True, stop=True)
                r0 = (ft % 2) * 64
                cbase = (ft // 2) * BL + q0
                dst = xp_all[r0:r0 + 50, cbase:cbase + QW]
                # Pool cannot read PSUM on TRN2 -> DVE does this copy
                nc.vector.tensor_copy(dst, xq[:])
                if fq in (0, 2):
                    e = 2 * ft + fq // 2
                    nc.vector.bn_stats(out=statsH[:, e, :],
                                       in_=fhn[:, q0:q0 + 512])
                    ex = 2 * (ft // 2) + fq // 2
                    nc.vector.bn_stats(
                        out=statsX[r0:r0 + 50, ex, :],
                        in_=xp_all[r0:r0 + 50, cbase:cbase + 512])

            hc, cc = h0, c0
            for t in range(T):
                hn = hpool.tile([HID, BL], BF16)
                cn = cpool.tile([HID, BL], BF16)
                for q in range(NQ):
                    q0 = q * QW
                    ga = gpa.tile([HID, QW], FP32, tag="ga")
                    nc.tensor.matmul(ga[:, 0:512], wct[:, GI * HID:(GI + 1) * HID],
                                     hc[:, q0:q0 + 512], start=True, stop=True)
                    nc.tensor.matmul(ga[:, 512:QW], wct[:, GI * HID:(GI + 1) * HID],
                                     hc[:, q0 + 512:q0 + QW], start=True, stop=True)
                    gb = gpb.tile([HID, QW], FP32, tag="gb")
                    nc.tensor.matmul(gb[:, 0:512], wct[:, GG * HID:(GG + 1) * HID],
                                     hc[:, q0:q0 + 512], start=True, stop=True)
                    nc.tensor.matmul(gb[:, 512:QW], wct[:, GG * HID:(GG + 1) * HID],
                                     hc[:, q0 + 512:q0 + QW], start=True, stop=True)
                    si = gt.tile([HID, QW], BF16, tag="si")
                    nc.scalar.activation(si[:], ga[:], AF.Sigmoid,
                                         bias=bct[:, GI:GI + 1])
                    tg = gt.tile([HID, QW], BF16, tag="tg")
                    nc.scalar.activation(tg[:], gb[:], AF.Tanh,
                                         bias=bct[:, GG:GG + 1])
                    ga2 = gpa.tile([HID, QW], FP32, tag="ga")
                    nc.tensor.matmul(ga2[:, 0:512], wct[:, GF * HID:(GF + 1) * HID],
                                     hc[:, q0:q0 + 512], start=True, stop=True)
                    nc.tensor.matmul(ga2[:, 512:QW], wct[:, GF * HID:(GF + 1) * HID],
                                     hc[:, q0 + 512:q0 + QW], start=True, stop=True)
                    gb2 = gpb.tile([HID, QW], FP32, tag="gb")
                    nc.tensor.matmul(gb2[:, 0:512], wct[:, GO * HID:(GO + 1) * HID],
                                     hc[:, q0:q0 + 512], start=True, stop=True)
                    nc.tensor.matmul(gb2[:, 512:QW], wct[:, GO * HID:(GO + 1) * HID],
                                     hc[:, q0 + 512:q0 + QW], start=True, stop=True)
                    # flush the (q-2) tail HERE so its DVE/PE ops land ahead
                    # of this quarter's t2/cn in the in-order engine queues
                    # (otherwise next quarter's gate matmul transitively
                    # waits on the Pool t1 mul)
                    if len(pending) > 1:
                        flush_one()
                    sf = gt.tile([HID, QW], BF16, tag="sf")
                    nc.scalar.activation(sf[:], ga2[:], AF.Sigmoid,
                                         bias=bct[:, GF:GF + 1])
                    so = gt.tile([HID, QW], BF16, tag="so", bufs=3)
                    nc.scalar.activation(so[:], gb2[:], AF.Sigmoid,
                                         bias=bct[:, GO:GO + 1])
                    # t2 = sig(i)*tanh(g)  (in place on tg, DVE bf16 2x)
                    nc.vector.tensor_mul(tg[:], si[:], tg[:])
                    # t1 = sig(f)*c        (in place on sf, Pool)
                    nc.gpsimd.tensor_mul(sf[:], sf[:], cc[:, q0:q0 + QW])
                    if ms_chunks:
                        mo, mw = ms_chunks.pop(0)
                        nc.gpsimd.memset(xp_all[32:64, mo:mo + mw], 0.0)
                    # c_new = t1 + t2      (DVE bf16 2x)
                    nc.vector.tensor_add(cn[:, q0:q0 + QW], sf[:], tg[:])
                    pending.append((t, q, so, hn, cn))
                hc, cc = hn, cn
            while pending:
                flush_one()
            if "h" in dbg:
                dh = scal.tile([HID, 512], FP32)
                nc.vector.tensor_copy(dh[:], hc[:, 0:512])
                nc.sync.dma_start(out=dbg_h12[:, :], in_=dh[:])

            # ---------------- stats finalize #1 (per-shard) -------------
            def mk_chain():
                ctr = [0]
                def mk():
                    ctr[0] += 1
                    return scal.tile([1, 1], FP32, name=f"sc{ctr[0]}",
                                     tag=f"sc{ctr[0]}")
                return mk
            mk = mk_chain()
            eps_t = scal.tile([1, 1], FP32)
            nc.vector.memset(eps_t[:], EPS)
            c15 = scal.tile([1, 1], FP32)
            nc.vector.memset(c15[:], 1.5)

            def rstd_of(v):
                """1/sqrt(v+eps); ACT sqrt + exact DVE reciprocal is ~1e-3
                accurate which is plenty for the 2e-2 gate"""
                rt = mk()
                nc.scalar.activation(rt[:], v[:], AF.Sqrt, bias=eps_t[0:1])
                r = mk(); nc.vector.reciprocal(r[:], rt[:])
                return r

            # channel-wise mean / E[x^2] of H and X
            mvH = scal.tile([HID, 2], FP32)
            nc.vector.bn_aggr(out=mvH[:], in_=statsH[:].rearrange(
                "p a b -> p (a b)"))
            e2H = scal.tile([HID, 1], FP32)
            nc.vector.tensor_mul(e2H[:], mvH[:, 0:1], mvH[:, 0:1])
            nc.vector.tensor_add(e2H[:], e2H[:], mvH[:, 1:2])
            mvX = scal.tile([XR, 2], FP32)
            nc.vector.bn_aggr(out=mvX[:], in_=statsX[:].rearrange(
                "p a b -> p (a b)"))
            e2X = scal.tile([XR, 1], FP32)
            nc.vector.tensor_mul(e2X[:], mvX[:, 0:1], mvX[:, 0:1])
            nc.vector.tensor_add(e2X[:], e2X[:], mvX[:, 1:2])

            # global BN1 stats via ones-matmul
            smat = scal.tile([HID, 2], FP32)
            nc.vector.tensor_copy(smat[:, 0:1], mvH[:, 0:1])
            nc.vector.tensor_copy(smat[:, 1:2], e2H[:])
            sp1 = gpa.tile([HID, QW], FP32, tag="ga")
            nc.tensor.matmul(sp1[0:1, 0:2], ones[:], smat[:],
                             start=True, stop=True)
            srow = scal.tile([1, 2], FP32)
            nc.vector.tensor_copy(srow[:], sp1[0:1, 0:2])

            m1 = mk(); nc.scalar.mul(m1[:], srow[:, 0:1], 1.0 / HID)
            E2 = mk(); nc.scalar.mul(E2[:], srow[:, 1:2], 1.0 / HID)
            msq = mk(); nc.vector.tensor_mul(msq[:], m1[:], m1[:])
            v1 = mk(); nc.vector.tensor_sub(v1[:], E2[:], msq[:])
            rstd1 = rstd_of(v1)
            a1 = mk(); nc.vector.tensor_mul(a1[:], rstd1[:], cst[:, 0:1])
            bb = mk(); nc.vector.tensor_mul(bb[:], m1[:], a1[:])
            nc.vector.tensor_sub(bb[:], cst[:, 1:2], bb[:])

            # broadcast bb to 100 partitions via PE; c1 = bb*s1 + b1
            bc_ps = gpa.tile([HID, QW], FP32, tag="ga")
            nc.tensor.matmul(bc_ps[0:XR, 0:1], ones_row[:, 0:XR], bb[:],
                             start=True, stop=True)
            bb_b = scal.tile([XR, 1], FP32)
            nc.vector.tensor_copy(bb_b[:], bc_ps[0:XR, 0:1])
            c1 = scal.tile([XR, 1], FP32)
            nc.vector.tensor_scalar(out=c1[:], in0=s1t[:], scalar1=bb_b[:],
                                    scalar2=b1t[:], op0=OP.mult, op1=OP.add)

            # global BN2 stats: x = a1*xp + c1
            smat2 = scal.tile([HID, 5], FP32)
            nc.vector.memset(smat2[:], 0.0)
            for lo, hi in ((0, 50), (64, XR)):
                s_ = slice(lo, hi)
                nc.vector.tensor_copy(smat2[s_, 0:1], mvX[s_, 0:1])
                nc.vector.tensor_copy(smat2[s_, 1:2], e2X[s_, :])
                nc.vector.tensor_copy(smat2[s_, 2:3], c1[s_, :])
                nc.vector.tensor_mul(smat2[s_, 3:4], c1[s_, :], mvX[s_, 0:1])
                nc.vector.tensor_mul(smat2[s_, 4:5], c1[s_, :], c1[s_, :])
            sp2 = gpa.tile([HID, QW], FP32, tag="ga")
            nc.tensor.matmul(sp2[0:1, 0:5], ones[:], smat2[:],
                             start=True, stop=True)
            srow2 = scal.tile([1, 5], FP32)
            nc.vector.tensor_copy(srow2[:], sp2[0:1, 0:5])

            # m2 = a1*mean(mX) + mean(c1)
            t1_ = mk(); nc.vector.tensor_mul(t1_[:], a1[:], srow2[:, 0:1])
            m2 = mk()
            nc.vector.tensor_add(m2[:], t1_[:], srow2[:, 2:3])
            nc.scalar.mul(m2[:], m2[:], 1.0 / 100.0)
            # E[x^2] = a1^2*mean(e2X) + 2*a1*mean(c1*mX) + mean(c1^2)
            a1sq = mk(); nc.vector.tensor_mul(a1sq[:], a1[:], a1[:])
            u1 = mk(); nc.vector.tensor_mul(u1[:], a1sq[:], srow2[:, 1:2])
            u2 = mk(); nc.vector.tensor_mul(u2[:], a1[:], srow2[:, 3:4])
            nc.scalar.mul(u2[:], u2[:], 2.0)
            nc.vector.tensor_add(u1[:], u1[:], u2[:])
            nc.vector.tensor_add(u1[:], u1[:], srow2[:, 4:5])
            E2x = mk(); nc.scalar.mul(E2x[:], u1[:], 1.0 / 100.0)
            m2sq = mk(); nc.vector.tensor_mul(m2sq[:], m2[:], m2[:])
            v2 = mk(); nc.vector.tensor_sub(v2[:], E2x[:], m2sq[:])
            rstd2 = rstd_of(v2)
            a2 = mk(); nc.vector.tensor_mul(a2[:], rstd2[:], cst[:, 2:3])
            b2a = mk(); nc.vector.tensor_mul(b2a[:], m2[:], a2[:])
            nc.vector.tensor_sub(b2a[:], cst[:, 3:4], b2a[:])
            A = mk(); nc.vector.tensor_mul(A[:], a2[:], a1[:])

            # broadcast (A, a2, b2a) to 100 partitions
            pk = scal.tile([1, 3], FP32)
            nc.vector.tensor_copy(pk[:, 0:1], A[:])
            nc.vector.tensor_copy(pk[:, 1:2], a2[:])
            nc.vector.tensor_copy(pk[:, 2:3], b2a[:])
            bc2 = gpa.tile([HID, QW], FP32, tag="ga")
            nc.tensor.matmul(bc2[0:XR, 0:3], ones_row[:, 0:XR], pk[:],
                             start=True, stop=True)
            bcs = scal.tile([XR, 3], FP32)
            nc.vector.tensor_copy(bcs[:], bc2[0:XR, 0:3])
            A_b = bcs[:, 0:1]
            Cv = scal.tile([XR, 1], FP32)        # C = a2*c1 + b2a
            nc.vector.tensor_scalar(out=Cv[:], in0=c1[:], scalar1=bcs[:, 1:2],
                                    scalar2=bcs[:, 2:3], op0=OP.mult,
                                    op1=OP.add)
            negC = scal.tile([XR, 1], FP32)
            nc.scalar.mul(negC[:], Cv[:], -1.0)

            if "q" in dbg:
                dsc = scal.tile([1, 16], FP32)
                nc.vector.memset(dsc[:], 0.0)
                for k_, v_ in enumerate([m1, v1, rstd1, a1, bb, m2, v2,
                                         rstd2, a2, b2a, A]):
                    nc.vector.tensor_copy(dsc[:, k_:k_ + 1], v_[:])
                nc.sync.dma_start(out=dbg_scal[:, :], in_=dsc[:])
            if "x" in dbg:
                dx = scal.tile([XR, 1024], FP32)
                nc.vector.tensor_copy(dx[:], xp_all[:, 0:1024])
                nc.sync.dma_start(out=dbg_xp[:, :], in_=dx[:])

            # ---------------- pass 2a: y' = max(A*xp, -C), stats ---------
            NCH = XC // 2048                     # 12 chunks
            for ch in range(NCH):
                cs = ch * 2048
                sl = xp_all[:, cs:cs + 2048]
                nc.vector.tensor_scalar(out=sl, in0=sl, scalar1=A_b,
                                        scalar2=negC[:], op0=OP.mult,
                                        op1=OP.max)
                if ch % 2 == 0:
                    nc.vector.bn_stats(out=statsY[:, ch // 2, :],
                                       in_=xp_all[:, cs:cs + 512])

            # ---------------- stats finalize #2 (BN3) --------------------
            mvY = scal.tile([XR, 2], FP32)
            nc.vector.bn_aggr(out=mvY[:], in_=statsY[:].rearrange(
                "p a b -> p (a b)"))
            e2Y = scal.tile([XR, 1], FP32)
            nc.vector.tensor_mul(e2Y[:], mvY[:, 0:1], mvY[:, 0:1])
            nc.vector.tensor_add(e2Y[:], e2Y[:], mvY[:, 1:2])
            # y = y' + C: mean/E2 shift
            smat3 = scal.tile([HID, 5], FP32)
            nc.vector.memset(smat3[:], 0.0)
            for lo, hi in ((0, 50), (64, XR)):
                s_ = slice(lo, hi)
                nc.vector.tensor_copy(smat3[s_, 0:1], mvY[s_, 0:1])
                nc.vector.tensor_copy(smat3[s_, 1:2], e2Y[s_, :])
                nc.vector.tensor_copy(smat3[s_, 2:3], Cv[s_, :])
                nc.vector.tensor_mul(smat3[s_, 3:4], Cv[s_, :], mvY[s_, 0:1])
                nc.vector.tensor_mul(smat3[s_, 4:5], Cv[s_, :], Cv[s_, :])
            sp3 = gpa.tile([HID, QW], FP32, tag="ga")
            nc.tensor.matmul(sp3[0:1, 0:5], ones[:], smat3[:],
                             start=True, stop=True)
            srow3 = scal.tile([1, 5], FP32)
            nc.vector.tensor_copy(srow3[:], sp3[0:1, 0:5])

            m3 = mk()
            nc.vector.tensor_add(m3[:], srow3[:, 0:1], srow3[:, 2:3])
            nc.scalar.mul(m3[:], m3[:], 1.0 / 100.0)
            w1_ = mk(); nc.scalar.mul(w1_[:], srow3[:, 3:4], 2.0)
            nc.vector.tensor_add(w1_[:], w1_[:], srow3[:, 1:2])
            nc.vector.tensor_add(w1_[:], w1_[:], srow3[:, 4:5])
            E3 = mk(); nc.scalar.mul(E3[:], w1_[:], 1.0 / 100.0)
            m3sq = mk(); nc.vector.tensor_mul(m3sq[:], m3[:], m3[:])
            v3 = mk(); nc.vector.tensor_sub(v3[:], E3[:], m3sq[:])
            rstd3 = rstd_of(v3)
            a3 = mk(); nc.vector.tensor_mul(a3[:], rstd3[:], cst[:, 4:5])
            b3a = mk(); nc.vector.tensor_mul(b3a[:], m3[:], a3[:])
            nc.vector.tensor_sub(b3a[:], cst[:, 5:6], b3a[:])

            # CW2[j] = sum_p C_p * W2blk[p, j]
            cw_ps = gpa.tile([HID, QW], FP32, tag="ga")
            nc.tensor.matmul(cw_ps[0:4, 0:1], w2tf[:], Cv[:],
                             start=True, stop=True)
            CW2 = scal.tile([4, 1], FP32)
            nc.vector.tensor_copy(CW2[:], cw_ps[0:4, 0:1])
            # broadcast a3, b3a to 4 partitions
            pk3 = scal.tile([1, 2], FP32)
            nc.vector.tensor_copy(pk3[:, 0:1], a3[:])
            nc.vector.tensor_copy(pk3[:, 1:2], b3a[:])
            bc3 = gpa.tile([HID, QW], FP32, tag="ga")
            nc.tensor.matmul(bc3[0:4, 0:2], ones_row[:, 0:4], pk3[:],
                             start=True, stop=True)
            ab3 = scal.tile([4, 2], FP32)
            nc.vector.tensor_copy(ab3[:], bc3[0:4, 0:2])
            a3_b = ab3[:, 0:1]
            # cbv = a3*CW2 + b3a*s2blk + b2blk
            cbv = scal.tile([4, 1], FP32)
            nc.vector.tensor_scalar(out=cbv[:], in0=s2t[:],
                                    scalar1=ab3[:, 1:2], scalar2=b2t[:],
                                    op0=OP.mult, op1=OP.add)
            t4 = scal.tile([4, 1], FP32)
            nc.vector.tensor_mul(t4[:], CW2[:], ab3[:, 0:1])
            nc.vector.tensor_add(cbv[:], cbv[:], t4[:])

            if "v" in dbg:
                dvc = scal.tile([XR, 4], FP32)
                nc.vector.tensor_copy(dvc[:, 0:1], c1[:])
                nc.vector.tensor_copy(dvc[:, 1:2], Cv[:])
                nc.vector.tensor_copy(dvc[:, 2:3], mvX[:, 0:1])
                nc.vector.tensor_copy(dvc[:, 3:4], e2X[:])
                nc.sync.dma_start(out=dbg_vec[:, :], in_=dvc[:])

            # ---------------- pass 2b: out = a3*(y' @ W2blk) + cbv -------
            # final affine+copy split across ACT/DVE/Pool so the tail is
            # paced by three engines instead of one
            psum_ctx.close()
            rp = ctx.enter_context(
                tc.tile_pool(name="rp", bufs=4, space="PSUM"))
            # engine rotation (Pool cannot read PSUM): ACT 7 : DVE 5
            ENG = [0, 1, 0, 1, 0, 1, 0, 0, 1, 0, 1, 0]
            # stage 4096-wide, DMA once per 4 affine chunks (HWDGE desc-gen
            # is ~625ns per dma_start — 24 small DMAs would pace the tail)
            stg = None
            for ch in range(XC // 1024):
                cs = ch * 1024
                r2_ = rp.tile([4, 1024], FP32, tag="rp")
                for j in range(2):
                    csl = slice(cs + j * 512, cs + (j + 1) * 512)
                    nc.tensor.matmul(
                        r2_[:, j * 512:(j + 1) * 512], w2t[:],
                        xp_all[:, csl], start=True, stop=True)
                if ch % 4 == 0:
                    stg = spool.tile([4, 4096], FP32, tag="os", bufs=2)
                so_ = stg[:, (ch % 4) * 1024:(ch % 4 + 1) * 1024]
                eng = ENG[ch % 12]
                if eng == 0:
                    nc.scalar.activation(so_, r2_[:], AF.Identity,
                                         bias=cbv[:], scale=a3_b)
                else:
                    nc.vector.tensor_scalar(out=so_, in0=r2_[:],
                                            scalar1=a3_b, scalar2=cbv[:],
                                            op0=OP.mult, op1=OP.add)
                if ch % 4 == 3:
                    nc.sync.dma_start(out=out_d[:, cs - 3072:cs + 1024],
                                      in_=stg[:])

    nc.finalize()
    return nc


_NC_CACHE = {}


def _get_nc(dbg=""):
    if dbg not in _NC_CACHE:
        _NC_CACHE[dbg] = build_nc(dbg)
    return _NC_CACHE[dbg]


def kernel(h, c, W_ih, W_hh, b_ih, b_hh, gamma1, beta1, gamma2, beta2,
           gamma3, beta3, W1, b1, W2, b2, dbg=""):
    h = np.asarray(h, np.float32)
    c = np.asarray(c, np.float32)
    W_ih = np.asarray(W_ih, np.float32)
    W_hh = np.asarray(W_hh, np.float32)
    b_ih = np.asarray(b_ih, np.float32)
    b_hh = np.asarray(b_hh, np.float32)
    W1 = np.asarray(W1, np.float32)
    b1 = np.asarray(b1, np.float32)
    W2 = np.asarray(W2, np.float32)
    b2 = np.asarray(b2, np.float32)
    bf = ml_dtypes.bfloat16

    hT = np.ascontiguousarray(h[0].T.astype(bf))          # [128, B] bf16
    cT = np.ascontiguousarray(c[0].T.astype(bf))
    Wc = W_ih + W_hh                                      # [512, 128]
    WcT = np.ascontiguousarray(Wc.T.astype(bf))           # [128, 512]
    bc = b_ih + b_hh
    bcT = np.ascontiguousarray(bc.reshape(4, HID).T)      # [128, 4] fp32
    W1T = np.ascontiguousarray(W1.T.astype(bf))           # [128, 50]
    # block-diagonal W2^T over 2 timesteps: [100, 4]
    W2B = np.zeros((114, 4), np.float32)
    W2B[0:50, 0:2] = W2.T
    W2B[64:114, 2:4] = W2.T
    s1 = W1.sum(1)                                        # [50]
    s1b = np.zeros((114, 1), np.float32)
    s1b[0:50, 0], s1b[64:114, 0] = s1, s1
    b1b = np.zeros((114, 1), np.float32)
    b1b[0:50, 0], b1b[64:114, 0] = b1, b1
    s2 = W2.sum(1)                                        # [2]
    s2b = np.ascontiguousarray(np.tile(s2, 2)[:, None])   # [4,1]
    b2b = np.ascontiguousarray(np.tile(b2, 2)[:, None])
    consts = np.array([[float(gamma1), float(beta1), float(gamma2),
                        float(beta2), float(gamma3), float(beta3), 0.0, 0.0]],
                      np.float32)

    shared = {"WcT": WcT, "bcT": bcT, "W1T": W1T,
              "W2B": W2B.astype(bf), "W2Bf": W2B,
              "s1b": s1b, "b1b": b1b, "s2b": s2b, "b2b": b2b,
              "consts": consts}
    in_maps = []
    for i in range(NCORES):
        s = slice(i * BL, (i + 1) * BL)
        in_maps.append({"hT": np.ascontiguousarray(hT[:, s]),
                        "cT": np.ascontiguousarray(cT[:, s]), **shared})

    nc = _get_nc(dbg)
    res = run_bass_kernel_spmd(nc, in_maps, list(range(NCORES)))
    global LAST_EXEC_NS
    if getattr(res, "exec_time_ns", None):
        LAST_EXEC_NS = res.exec_time_ns
    if dbg:
        return res

    out = np.empty((B, T, 2), np.float32)
    for i in range(NCORES):
        arr = res.results[i]["out"]              # [4, 24576]
        # row = (t%2)*2 + ch ; col = (t//2)*4096 + b
        a4 = arr.reshape(2, 2, T // 2, BL)       # [parity, ch, pair, b]
        out[i * BL:(i + 1) * BL] = a4.transpose(3, 2, 0, 1).reshape(BL, T, 2)
    return out


# revision 5
# speedup vs baseline: 1.0282x; 1.0131x over previous
"""Trainium2 Bass kernel v2 for nn_Decoder (12-step LSTM cell + BN/Linear
head), data-parallel over batch across 8 NeuronCores.

Key differences vs v1 baseline:
  * all big matmuls in bf16 (1 PE cycle/row vs 4 for fp32)
  * per-shard BN statistics (sharding hint allows it; stat error ~1e-3,
    tolerance 2e-2) -> no collectives at all
  * elementwise chain in bf16: DVE runs its TensorTensor/TensorScalar ops
    in 2x/4x perf modes
  * xp (the pure H @ W1.T partial) kept in SBUF in bf16, packed 2 steps
    deep: [114, 24576] (rows 0-49 even steps, 64-113 odd steps; engine
    partition bases must be 0/32/64/96) -> phase-2 passes have half the
    per-partition free size, and W2 is applied as a block-diagonal matmul
  * BN stats subsampled 1/4 (bn_stats hw max 512 free anyway)
  * relu folded into a DVE tensor_scalar: y' = max(A*xp, -C); the +C shift
    is pushed into the final output bias analytically
  * engine balance: ACT = 5 transcendental passes/step (bottleneck);
    DVE = t2/cn/hn muls + stats; Pool = t1 mul + xp PSUM->SBUF copy

Math recap (see v1 docstring): Wc = W_ih + W_hh, bc = b_ih + b_hh,
12 steps of z = Wc@h + bc; c' = sig(f)c + sig(i)tanh(g); h' = sig(o)tanh(c').
BN1/BN2/BN3 are whole-tensor-stat batchnorms -> scalar affine transforms,
derived per shard from (subsampled) channel stats.
"""

import sys

sys.path.insert(0, "/opt/trn_rl_repo")

import numpy as np
import ml_dtypes

import concourse.bass as bass
import concourse.mybir as mybir
import concourse.tile as tile
from concourse import bacc
from concourse.bass_utils import run_bass_kernel_spmd

AF = mybir.ActivationFunctionType
OP = mybir.AluOpType
FP32 = mybir.dt.float32
BF16 = mybir.dt.bfloat16

B = 32768
HID = 128
T = 12
NCORES = 8
BL = B // NCORES            # 4096 batch per core
QW = 1024                   # cols per quarter
NQ = BL // QW               # 4 quarters
XC = (T // 2) * BL          # 24576 cols of the packed xp buffer
XR = 114                    # xp partition rows: 0-49 even t, 64-113 odd t
                            # (engine partition base must be 0/32/64/96)
EPS = 1e-5
LAST_EXEC_NS = None

# gate order in the PyTorch weight layout: i, f, g, o
GI, GF, GG, GO = 0, 1, 2, 3


def build_nc(dbg=""):
    nc = bacc.Bacc(None, target_bir_lowering=False, debug=False)

    # ---------------- I/O ----------------
    hT = nc.dram_tensor("hT", [HID, BL], BF16, kind="ExternalInput")
    cT = nc.dram_tensor("cT", [HID, BL], BF16, kind="ExternalInput")
    WcT = nc.dram_tensor("WcT", [HID, 4 * HID], BF16, kind="ExternalInput")
    bcT = nc.dram_tensor("bcT", [HID, 4], FP32, kind="ExternalInput")
    W1T = nc.dram_tensor("W1T", [HID, 50], BF16, kind="ExternalInput")
    W2B = nc.dram_tensor("W2B", [XR, 4], BF16, kind="ExternalInput")
    W2Bf = nc.dram_tensor("W2Bf", [XR, 4], FP32, kind="ExternalInput")
    s1b = nc.dram_tensor("s1b", [XR, 1], FP32, kind="ExternalInput")
    b1b = nc.dram_tensor("b1b", [XR, 1], FP32, kind="ExternalInput")
    s2b = nc.dram_tensor("s2b", [4, 1], FP32, kind="ExternalInput")
    b2b = nc.dram_tensor("b2b", [4, 1], FP32, kind="ExternalInput")
    consts = nc.dram_tensor("consts", [1, 8], FP32, kind="ExternalInput")
    out_d = nc.dram_tensor("out", [4, XC], FP32, kind="ExternalOutput")
    if "x" in dbg:
        dbg_xp = nc.dram_tensor("dbg_xp", [XR, 1024], FP32,
                                kind="ExternalOutput")
    if "h" in dbg:
        dbg_h1 = nc.dram_tensor("dbg_h1", [HID, 512], FP32,
                                kind="ExternalOutput")
        dbg_h12 = nc.dram_tensor("dbg_h12", [HID, 512], FP32,
                                 kind="ExternalOutput")
    if "q" in dbg:
        dbg_scal = nc.dram_tensor("dbg_scal", [1, 16], FP32,
                                  kind="ExternalOutput")
    if "v" in dbg:
        dbg_vec = nc.dram_tensor("dbg_vec", [XR, 4], FP32,
                                 kind="ExternalOutput")

    with tile.TileContext(nc) as tc:
        import contextlib
        ctx = contextlib.ExitStack()
        with ctx:
            singles = ctx.enter_context(tc.tile_pool(name="singles", bufs=1))
            hpool = ctx.enter_context(tc.tile_pool(name="h", bufs=2))
            cpool = ctx.enter_context(tc.tile_pool(name="c", bufs=2))
            gt = ctx.enter_context(tc.tile_pool(name="gates", bufs=2))
            scal = ctx.enter_context(tc.tile_pool(name="scal", bufs=1))
            spool = ctx.enter_context(tc.tile_pool(name="stage", bufs=2))
            psum_ctx = contextlib.ExitStack()
            gpa = psum_ctx.enter_context(
                tc.tile_pool(name="gpa", bufs=1, space="PSUM"))
            gpb = psum_ctx.enter_context(
                tc.tile_pool(name="gpb", bufs=1, space="PSUM"))
            xf = psum_ctx.enter_context(
                tc.tile_pool(name="xf", bufs=2, space="PSUM"))

            # ---------------- loads (critical path first) ----------------
            wct = singles.tile([HID, 4 * HID], BF16)
            nc.sync.dma_start(out=wct[:], in_=WcT[:, :])
            bct = singles.tile([HID, 4], FP32)
            nc.sync.dma_start(out=bct[:], in_=bcT[:, :])
            h0 = hpool.tile([HID, BL], BF16)
            c0 = cpool.tile([HID, BL], BF16)
            for k in range(8):
                s = slice(k * 512, (k + 1) * 512)
                nc.sync.dma_start(out=h0[:, s], in_=hT[:, s])
                nc.sync.dma_start(out=c0[:, s], in_=cT[:, s])
            w1t = singles.tile([HID, 50], BF16)
            nc.sync.dma_start(out=w1t[:], in_=W1T[:, :])
            w2t = singles.tile([XR, 4], BF16)
            nc.sync.dma_start(out=w2t[:], in_=W2B[:, :])
            w2tf = singles.tile([XR, 4], FP32)
            nc.sync.dma_start(out=w2tf[:], in_=W2Bf[:, :])
            s1t = singles.tile([XR, 1], FP32)
            nc.sync.dma_start(out=s1t[:], in_=s1b[:, :])
            b1t = singles.tile([XR, 1], FP32)
            nc.sync.dma_start(out=b1t[:], in_=b1b[:, :])
            s2t = singles.tile([4, 1], FP32)
            nc.sync.dma_start(out=s2t[:], in_=s2b[:, :])
            b2t = singles.tile([4, 1], FP32)
            nc.sync.dma_start(out=b2t[:], in_=b2b[:, :])
            cst = singles.tile([1, 8], FP32)
            nc.sync.dma_start(out=cst[:], in_=consts[:, :])
            ones = singles.tile([HID, 1], FP32)
            nc.vector.memset(ones[:], 1.0)
            ones_row = singles.tile([1, HID], FP32)
            nc.vector.memset(ones_row[:], 1.0)

            xp_all = singles.tile([XR, XC], BF16)
            statsH = singles.tile([HID, 2 * T, 6], FP32)
            statsX = singles.tile([XR, T, 6], FP32)
            statsY = singles.tile([XR, T // 2, 6], FP32)
            # dead partition rows 50-63 are never written by the packed
            # stores but ARE covered by full-[XR] reads -> zero them.
            # The xp memset goes on Pool in per-quarter chunks (see the
            # step loop) so it doesn't head-of-line-block Pool's t1 muls.
            nc.vector.memset(statsX[32:64, :, :], 0.0)
            nc.vector.memset(statsY[32:64, :, :], 0.0)
            ms_chunks = [(k * 2048, 2048) for k in range(XC // 2048)]

            # ---------------- LSTM ----------------
            # Deferred "flush" (tanh(c')/h'/xp) runs 2 quarters behind the
            # gate pipeline and crosses step boundaries, so ACT never stalls
            # at the end of a step waiting on the Pool->DVE c' chain.
            pending = []

            def flush_one():
                ft, fq, fso, fhn, fcn = pending.pop(0)
                q0 = fq * QW
                tcn = gt.tile([HID, QW], BF16, tag="tcn")
                nc.scalar.activation(tcn[:], fcn[:, q0:q0 + QW], AF.Tanh)
                nc.vector.tensor_mul(fhn[:, q0:q0 + QW], fso[:], tcn[:])
                xq = xf.tile([50, QW], FP32, tag="xf")
                nc.tensor.matmul(xq[:, 0:512], w1t[:],
                                 fhn[:, q0:q0 + 512], start=True, stop=True)
                nc.tensor.matmul(xq[:, 512:QW], w1t[:],
                                 fhn[:, q0 + 512:q0 + QW],
                                 start=True, stop=True)
                r0 = (ft % 2) * 64
                cbase = (ft // 2) * BL + q0
                dst = xp_all[r0:r0 + 50, cbase:cbase + QW]
                # Pool cannot read PSUM on TRN2 -> DVE does this copy.
                # For the last step's drain-time flushes use ACT instead:
                # ACT idles in the drain while DVE is the straggler.
                if ft == T - 1 and fq >= 2:
                    nc.scalar.copy(dst, xq[:])
                else:
                    nc.vector.tensor_copy(dst, xq[:])
                if fq in (0, 2):
                    e = 2 * ft + fq // 2
                    nc.vector.bn_stats(out=statsH[:, e, :],
                                       in_=fhn[:, q0:q0 + 512])
                    ex = 2 * (ft // 2) + fq // 2
                    nc.vector.bn_stats(
                        out=statsX[r0:r0 + 50, ex, :],
                        in_=xp_all[r0:r0 + 50, cbase:cbase + 512])

            hc, cc = h0, c0
            for t in range(T):
                hn = hpool.tile([HID, BL], BF16)
                cn = cpool.tile([HID, BL], BF16)
                for q in range(NQ):
                    q0 = q * QW
                    ga = gpa.tile([HID, QW], FP32, tag="ga")
                    nc.tensor.matmul(ga[:, 0:512], wct[:, GI * HID:(GI + 1) * HID],
                                     hc[:, q0:q0 + 512], start=True, stop=True)
                    nc.tensor.matmul(ga[:, 512:QW], wct[:, GI * HID:(GI + 1) * HID],
                                     hc[:, q0 + 512:q0 + QW], start=True, stop=True)
                    gb = gpb.tile([HID, QW], FP32, tag="gb")
                    nc.tensor.matmul(gb[:, 0:512], wct[:, GG * HID:(GG + 1) * HID],
                                     hc[:, q0:q0 + 512], start=True, stop=True)
                    nc.tensor.matmul(gb[:, 512:QW], wct[:, GG * HID:(GG + 1) * HID],
                                     hc[:, q0 + 512:q0 + QW], start=True, stop=True)
                    si = gt.tile([HID, QW], BF16, tag="si")
                    nc.scalar.activation(si[:], ga[:], AF.Sigmoid,
                                         bias=bct[:, GI:GI + 1])
                    tg = gt.tile([HID, QW], BF16, tag="tg")
                    nc.scalar.activation(tg[:], gb[:], AF.Tanh,
                                         bias=bct[:, GG:GG + 1])
                    ga2 = gpa.tile([HID, QW], FP32, tag="ga")
                    nc.tensor.matmul(ga2[:, 0:512], wct[:, GF * HID:(GF + 1) * HID],
                                     hc[:, q0:q0 + 512], start=True, stop=True)
                    nc.tensor.matmul(ga2[:, 512:QW], wct[:, GF * HID:(GF + 1) * HID],
                                     hc[:, q0 + 512:q0 + QW], start=True, stop=True)
                    gb2 = gpb.tile([HID, QW], FP32, tag="gb")
                    nc.tensor.matmul(gb2[:, 0:512], wct[:, GO * HID:(GO + 1) * HID],
                                     hc[:, q0:q0 + 512], start=True, stop=True)
                    nc.tensor.matmul(gb2[:, 512:QW], wct[:, GO * HID:(GO + 1) * HID],
                                     hc[:, q0 + 512:q0 + QW], start=True, stop=True)
                    # flush the (q-2) tail HERE so its DVE/PE ops land ahead
                    # of this quarter's t2/cn in the in-order engine queues
                    # (otherwise next quarter's gate matmul transitively
                    # waits on the Pool t1 mul)
                    if len(pending) > 1:
                        flush_one()
                    sf = gt.tile([HID, QW], BF16, tag="sf")
                    nc.scalar.activation(sf[:], ga2[:], AF.Sigmoid,
                                         bias=bct[:, GF:GF + 1])
                    so = gt.tile([HID, QW], BF16, tag="so", bufs=3)
                    nc.scalar.activation(so[:], gb2[:], AF.Sigmoid,
                                         bias=bct[:, GO:GO + 1])
                    # t2 = sig(i)*tanh(g)  (in place on tg, DVE bf16 2x)
                    nc.vector.tensor_mul(tg[:], si[:], tg[:])
                    # t1 = sig(f)*c        (in place on sf, Pool)
                    nc.gpsimd.tensor_mul(sf[:], sf[:], cc[:, q0:q0 + QW])
                    if ms_chunks:
                        mo, mw = ms_chunks.pop(0)
                        nc.gpsimd.memset(xp_all[32:64, mo:mo + mw], 0.0)
                    # c_new = t1 + t2      (DVE bf16 2x)
                    nc.vector.tensor_add(cn[:, q0:q0 + QW], sf[:], tg[:])
                    pending.append((t, q, so, hn, cn))
                hc, cc = hn, cn
            while pending:
                flush_one()
            if "h" in dbg:
                dh = scal.tile([HID, 512], FP32)
                nc.vector.tensor_copy(dh[:], hc[:, 0:512])
                nc.sync.dma_start(out=dbg_h12[:, :], in_=dh[:])

            # ---------------- stats finalize #1 (per-shard) -------------
            def mk_chain():
                ctr = [0]
                def mk():
                    ctr[0] += 1
                    return scal.tile([1, 1], FP32, name=f"sc{ctr[0]}",
                                     tag=f"sc{ctr[0]}")
                return mk
            mk = mk_chain()
            eps_t = scal.tile([1, 1], FP32)
            nc.vector.memset(eps_t[:], EPS)
            c15 = scal.tile([1, 1], FP32)
            nc.vector.memset(c15[:], 1.5)

            def rstd_of(v):
                """1/sqrt(v+eps); ACT sqrt + exact DVE reciprocal is ~1e-3
                accurate which is plenty for the 2e-2 gate"""
                rt = mk()
                nc.scalar.activation(rt[:], v[:], AF.Sqrt, bias=eps_t[0:1])
                r = mk(); nc.vector.reciprocal(r[:], rt[:])
                return r

            # channel-wise mean / E[x^2] of H and X
            mvH = scal.tile([HID, 2], FP32)
            nc.vector.bn_aggr(out=mvH[:], in_=statsH[:].rearrange(
                "p a b -> p (a b)"))
            e2H = scal.tile([HID, 1], FP32)
            nc.vector.tensor_mul(e2H[:], mvH[:, 0:1], mvH[:, 0:1])
            nc.vector.tensor_add(e2H[:], e2H[:], mvH[:, 1:2])
            mvX = scal.tile([XR, 2], FP32)
            nc.vector.bn_aggr(out=mvX[:], in_=statsX[:].rearrange(
                "p a b -> p (a b)"))
            e2X = scal.tile([XR, 1], FP32)
            nc.vector.tensor_mul(e2X[:], mvX[:, 0:1], mvX[:, 0:1])
            nc.vector.tensor_add(e2X[:], e2X[:], mvX[:, 1:2])

            # global BN1 stats via ones-matmul
            smat = scal.tile([HID, 2], FP32)
            nc.vector.tensor_copy(smat[:, 0:1], mvH[:, 0:1])
            nc.vector.tensor_copy(smat[:, 1:2], e2H[:])
            sp1 = gpa.tile([HID, QW], FP32, tag="ga")
            nc.tensor.matmul(sp1[0:1, 0:2], ones[:], smat[:],
                             start=True, stop=True)
            srow = scal.tile([1, 2], FP32)
            nc.vector.tensor_copy(srow[:], sp1[0:1, 0:2])

            m1 = mk(); nc.scalar.mul(m1[:], srow[:, 0:1], 1.0 / HID)
            E2 = mk(); nc.scalar.mul(E2[:], srow[:, 1:2], 1.0 / HID)
            msq = mk(); nc.vector.tensor_mul(msq[:], m1[:], m1[:])
            v1 = mk(); nc.vector.tensor_sub(v1[:], E2[:], msq[:])
            rstd1 = rstd_of(v1)
            a1 = mk(); nc.vector.tensor_mul(a1[:], rstd1[:], cst[:, 0:1])
            bb = mk(); nc.vector.tensor_mul(bb[:], m1[:], a1[:])
            nc.vector.tensor_sub(bb[:], cst[:, 1:2], bb[:])

            # broadcast bb to 100 partitions via PE; c1 = bb*s1 + b1
            bc_ps = gpa.tile([HID, QW], FP32, tag="ga")
            nc.tensor.matmul(bc_ps[0:XR, 0:1], ones_row[:, 0:XR], bb[:],
                             start=True, stop=True)
            bb_b = scal.tile([XR, 1], FP32)
            nc.vector.tensor_copy(bb_b[:], bc_ps[0:XR, 0:1])
            c1 = scal.tile([XR, 1], FP32)
            nc.vector.tensor_scalar(out=c1[:], in0=s1t[:], scalar1=bb_b[:],
                                    scalar2=b1t[:], op0=OP.mult, op1=OP.add)

            # global BN2 stats: x = a1*xp + c1
            smat2 = scal.tile([HID, 5], FP32)
            nc.vector.memset(smat2[:], 0.0)
            for lo, hi in ((0, 50), (64, XR)):
                s_ = slice(lo, hi)
                nc.vector.tensor_copy(smat2[s_, 0:1], mvX[s_, 0:1])
                nc.vector.tensor_copy(smat2[s_, 1:2], e2X[s_, :])
                nc.vector.tensor_copy(smat2[s_, 2:3], c1[s_, :])
                nc.vector.tensor_mul(smat2[s_, 3:4], c1[s_, :], mvX[s_, 0:1])
                nc.vector.tensor_mul(smat2[s_, 4:5], c1[s_, :], c1[s_, :])
            sp2 = gpa.tile([HID, QW], FP32, tag="ga")
            nc.tensor.matmul(sp2[0:1, 0:5], ones[:], smat2[:],
                             start=True, stop=True)
            srow2 = scal.tile([1, 5], FP32)
            nc.vector.tensor_copy(srow2[:], sp2[0:1, 0:5])

            # m2 = a1*mean(mX) + mean(c1)
            t1_ = mk(); nc.vector.tensor_mul(t1_[:], a1[:], srow2[:, 0:1])
            m2 = mk()
            nc.vector.tensor_add(m2[:], t1_[:], srow2[:, 2:3])
            nc.scalar.mul(m2[:], m2[:], 1.0 / 100.0)
            # E[x^2] = a1^2*mean(e2X) + 2*a1*mean(c1*mX) + mean(c1^2)
            a1sq = mk(); nc.vector.tensor_mul(a1sq[:], a1[:], a1[:])
            u1 = mk(); nc.vector.tensor_mul(u1[:], a1sq[:], srow2[:, 1:2])
            u2 = mk(); nc.vector.tensor_mul(u2[:], a1[:], srow2[:, 3:4])
            nc.scalar.mul(u2[:], u2[:], 2.0)
            nc.vector.tensor_add(u1[:], u1[:], u2[:])
            nc.vector.tensor_add(u1[:], u1[:], srow2[:, 4:5])
            E2x = mk(); nc.scalar.mul(E2x[:], u1[:], 1.0 / 100.0)
            m2sq = mk(); nc.vector.tensor_mul(m2sq[:], m2[:], m2[:])
            v2 = mk(); nc.vector.tensor_sub(v2[:], E2x[:], m2sq[:])
            rstd2 = rstd_of(v2)
            a2 = mk(); nc.vector.tensor_mul(a2[:], rstd2[:], cst[:, 2:3])
            b2a = mk(); nc.vector.tensor_mul(b2a[:], m2[:], a2[:])
            nc.vector.tensor_sub(b2a[:], cst[:, 3:4], b2a[:])
            A = mk(); nc.vector.tensor_mul(A[:], a2[:], a1[:])

            # broadcast (A, a2, b2a) to 100 partitions
            pk = scal.tile([1, 3], FP32)
            nc.vector.tensor_copy(pk[:, 0:1], A[:])
            nc.vector.tensor_copy(pk[:, 1:2], a2[:])
            nc.vector.tensor_copy(pk[:, 2:3], b2a[:])
            bc2 = gpa.tile([HID, QW], FP32, tag="ga")
            nc.tensor.matmul(bc2[0:XR, 0:3], ones_row[:, 0:XR], pk[:],
                             start=True, stop=True)
            bcs = scal.tile([XR, 3], FP32)
            nc.vector.tensor_copy(bcs[:], bc2[0:XR, 0:3])
            A_b = bcs[:, 0:1]
            Cv = scal.tile([XR, 1], FP32)        # C = a2*c1 + b2a
            nc.vector.tensor_scalar(out=Cv[:], in0=c1[:], scalar1=bcs[:, 1:2],
                                    scalar2=bcs[:, 2:3], op0=OP.mult,
                                    op1=OP.add)
            negC = scal.tile([XR, 1], FP32)
            nc.scalar.mul(negC[:], Cv[:], -1.0)
            # CW2[j] = sum_p C_p * W2blk[p, j] (hoisted: only needs Cv)
            cw_ps = gpa.tile([HID, QW], FP32, tag="ga")
            nc.tensor.matmul(cw_ps[0:4, 0:1], w2tf[:], Cv[:],
                             start=True, stop=True)
            CW2 = scal.tile([4, 1], FP32)
            nc.vector.tensor_copy(CW2[:], cw_ps[0:4, 0:1])

            if "q" in dbg:
                dsc = scal.tile([1, 16], FP32)
                nc.vector.memset(dsc[:], 0.0)
                for k_, v_ in enumerate([m1, v1, rstd1, a1, bb, m2, v2,
                                         rstd2, a2, b2a, A]):
                    nc.vector.tensor_copy(dsc[:, k_:k_ + 1], v_[:])
                nc.sync.dma_start(out=dbg_scal[:, :], in_=dsc[:])
            if "x" in dbg:
                dx = scal.tile([XR, 1024], FP32)
                nc.vector.tensor_copy(dx[:], xp_all[:, 0:1024])
                nc.sync.dma_start(out=dbg_xp[:, :], in_=dx[:])

            # ---------------- pass 2a: y' = max(A*xp, -C), stats ---------
            NCH = XC // 2048                     # 12 chunks
            # ACT (idle in this window) takes some chunks as a direct
            # relu(A*xp + C) = y; DVE chunks hold y' = y - C.  The final
            # affine bias differs per chunk class (see cbv_y / cbv_yp).
            R1ACT = {1, 5, 7, 11}
            for ch in range(NCH):
                cs = ch * 2048
                sl = xp_all[:, cs:cs + 2048]
                if ch in R1ACT:
                    nc.scalar.activation(sl, sl, AF.Relu, bias=Cv[:],
                                         scale=A_b)
                else:
                    nc.vector.tensor_scalar(out=sl, in0=sl, scalar1=A_b,
                                            scalar2=negC[:], op0=OP.mult,
                                            op1=OP.max)
                    if ch % 2 == 0:
                        nc.vector.bn_stats(out=statsY[:, ch // 2, :],
                                           in_=xp_all[:, cs:cs + 512])

            # ---------------- stats finalize #2 (BN3) --------------------
            mvY = scal.tile([XR, 2], FP32)
            nc.vector.bn_aggr(out=mvY[:], in_=statsY[:].rearrange(
                "p a b -> p (a b)"))
            e2Y = scal.tile([XR, 1], FP32)
            nc.vector.tensor_mul(e2Y[:], mvY[:, 0:1], mvY[:, 0:1])
            nc.vector.tensor_add(e2Y[:], e2Y[:], mvY[:, 1:2])
            # y = y' + C: mean/E2 shift
            smat3 = scal.tile([HID, 5], FP32)
            nc.vector.memset(smat3[:], 0.0)
            for lo, hi in ((0, 50), (64, XR)):
                s_ = slice(lo, hi)
                nc.vector.tensor_copy(smat3[s_, 0:1], mvY[s_, 0:1])
                nc.vector.tensor_copy(smat3[s_, 1:2], e2Y[s_, :])
                nc.vector.tensor_copy(smat3[s_, 2:3], Cv[s_, :])
                nc.vector.tensor_mul(smat3[s_, 3:4], Cv[s_, :], mvY[s_, 0:1])
                nc.vector.tensor_mul(smat3[s_, 4:5], Cv[s_, :], Cv[s_, :])
            sp3 = gpa.tile([HID, QW], FP32, tag="ga")
            nc.tensor.matmul(sp3[0:1, 0:5], ones[:], smat3[:],
                             start=True, stop=True)
            srow3 = scal.tile([1, 5], FP32)
            nc.vector.tensor_copy(srow3[:], sp3[0:1, 0:5])

            m3 = mk()
            nc.vector.tensor_add(m3[:], srow3[:, 0:1], srow3[:, 2:3])
            nc.scalar.mul(m3[:], m3[:], 1.0 / 100.0)
            w1_ = mk(); nc.scalar.mul(w1_[:], srow3[:, 3:4], 2.0)
            nc.vector.tensor_add(w1_[:], w1_[:], srow3[:, 1:2])
            nc.vector.tensor_add(w1_[:], w1_[:], srow3[:, 4:5])
            E3 = mk(); nc.scalar.mul(E3[:], w1_[:], 1.0 / 100.0)
            m3sq = mk(); nc.vector.tensor_mul(m3sq[:], m3[:], m3[:])
            v3 = mk(); nc.vector.tensor_sub(v3[:], E3[:], m3sq[:])
            rstd3 = rstd_of(v3)
            a3 = mk(); nc.vector.tensor_mul(a3[:], rstd3[:], cst[:, 4:5])
            b3a = mk(); nc.vector.tensor_mul(b3a[:], m3[:], a3[:])
            nc.vector.tensor_sub(b3a[:], cst[:, 5:6], b3a[:])

            # broadcast a3, b3a to 4 partitions
            pk3 = scal.tile([1, 2], FP32)
            nc.vector.tensor_copy(pk3[:, 0:1], a3[:])
            nc.vector.tensor_copy(pk3[:, 1:2], b3a[:])
            bc3 = gpa.tile([HID, QW], FP32, tag="ga")
            nc.tensor.matmul(bc3[0:4, 0:2], ones_row[:, 0:4], pk3[:],
                             start=True, stop=True)
            ab3 = scal.tile([4, 2], FP32)
            nc.vector.tensor_copy(ab3[:], bc3[0:4, 0:2])
            a3_b = ab3[:, 0:1]
            # cbv_y  = b3a*s2blk + b2blk          (chunks holding y)
            # cbv_yp = cbv_y + a3*CW2              (chunks holding y'=y-C)
            cbv_y = scal.tile([4, 1], FP32)
            nc.vector.tensor_scalar(out=cbv_y[:], in0=s2t[:],
                                    scalar1=ab3[:, 1:2], scalar2=b2t[:],
                                    op0=OP.mult, op1=OP.add)
            t4 = scal.tile([4, 1], FP32)
            nc.vector.tensor_mul(t4[:], CW2[:], ab3[:, 0:1])
            cbv_yp = scal.tile([4, 1], FP32)
            nc.vector.tensor_add(cbv_yp[:], cbv_y[:], t4[:])

            if "v" in dbg:
                dvc = scal.tile([XR, 4], FP32)
                nc.vector.tensor_copy(dvc[:, 0:1], c1[:])
                nc.vector.tensor_copy(dvc[:, 1:2], Cv[:])
                nc.vector.tensor_copy(dvc[:, 2:3], mvX[:, 0:1])
                nc.vector.tensor_copy(dvc[:, 3:4], e2X[:])
                nc.sync.dma_start(out=dbg_vec[:, :], in_=dvc[:])

            # ---------------- pass 2b: out = a3*(y' @ W2blk) + cbv -------
            # final affine+copy split across ACT/DVE/Pool so the tail is
            # paced by three engines instead of one
            psum_ctx.close()
            rp = ctx.enter_context(
                tc.tile_pool(name="rp", bufs=4, space="PSUM"))
            # engine rotation (Pool cannot read PSUM): ACT 7 : DVE 5
            ENG = [0, 1, 0, 1, 0, 1, 0, 0, 1, 0, 1, 0]
            # stage 4096-wide, DMA once per 4 affine chunks (HWDGE desc-gen
            # is ~625ns per dma_start — 24 small DMAs would pace the tail)
            stg = None
            for ch in range(XC // 1024):
                cs = ch * 1024
                r2_ = rp.tile([4, 1024], FP32, tag="rp")
                for j in range(2):
                    csl = slice(cs + j * 512, cs + (j + 1) * 512)
                    nc.tensor.matmul(
                        r2_[:, j * 512:(j + 1) * 512], w2t[:],
                        xp_all[:, csl], start=True, stop=True)
                if ch % 4 == 0:
                    stg = spool.tile([4, 4096], FP32, tag="os", bufs=2)
                so_ = stg[:, (ch % 4) * 1024:(ch % 4 + 1) * 1024]
                eng = ENG[ch % 12]
                cbv = cbv_y if (ch // 2) in R1ACT else cbv_yp
                if eng == 0:
                    nc.scalar.activation(so_, r2_[:], AF.Identity,
                                         bias=cbv[:], scale=a3_b)
                else:
                    nc.vector.tensor_scalar(out=so_, in0=r2_[:],
                                            scalar1=a3_b, scalar2=cbv[:],
                                            op0=OP.mult, op1=OP.add)
                if ch % 4 == 3:
                    nc.sync.dma_start(out=out_d[:, cs - 3072:cs + 1024],
                                      in_=stg[:])

    nc.finalize()
    return nc


_NC_CACHE = {}


def _get_nc(dbg=""):
    if dbg not in _NC_CACHE:
        _NC_CACHE[dbg] = build_nc(dbg)
    return _NC_CACHE[dbg]


def kernel(h, c, W_ih, W_hh, b_ih, b_hh, gamma1, beta1, gamma2, beta2,
           gamma3, beta3, W1, b1, W2, b2, dbg=""):
    h = np.asarray(h, np.float32)
    c = np.asarray(c, np.float32)
    W_ih = np.asarray(W_ih, np.float32)
    W_hh = np.asarray(W_hh, np.float32)
    b_ih = np.asarray(b_ih, np.float32)
    b_hh = np.asarray(b_hh, np.float32)
    W1 = np.asarray(W1, np.float32)
    b1 = np.asarray(b1, np.float32)
    W2 = np.asarray(W2, np.float32)
    b2 = np.asarray(b2, np.float32)
    bf = ml_dtypes.bfloat16

    hT = np.ascontiguousarray(h[0].T.astype(bf))          # [128, B] bf16
    cT = np.ascontiguousarray(c[0].T.astype(bf))
    Wc = W_ih + W_hh                                      # [512, 128]
    WcT = np.ascontiguousarray(Wc.T.astype(bf))           # [128, 512]
    bc = b_ih + b_hh
    bcT = np.ascontiguousarray(bc.reshape(4, HID).T)      # [128, 4] fp32
    W1T = np.ascontiguousarray(W1.T.astype(bf))           # [128, 50]
    # block-diagonal W2^T over 2 timesteps: [100, 4]
    W2B = np.zeros((114, 4), np.float32)
    W2B[0:50, 0:2] = W2.T
    W2B[64:114, 2:4] = W2.T
    s1 = W1.sum(1)                                        # [50]
    s1b = np.zeros((114, 1), np.float32)
    s1b[0:50, 0], s1b[64:114, 0] = s1, s1
    b1b = np.zeros((114, 1), np.float32)
    b1b[0:50, 0], b1b[64:114, 0] = b1, b1
    s2 = W2.sum(1)                                        # [2]
    s2b = np.ascontiguousarray(np.tile(s2, 2)[:, None])   # [4,1]
    b2b = np.ascontiguousarray(np.tile(b2, 2)[:, None])
    consts = np.array([[float(gamma1), float(beta1), float(gamma2),
                        float(beta2), float(gamma3), float(beta3), 0.0, 0.0]],
                      np.float32)

    shared = {"WcT": WcT, "bcT": bcT, "W1T": W1T,
              "W2B": W2B.astype(bf), "W2Bf": W2B,
              "s1b": s1b, "b1b": b1b, "s2b": s2b, "b2b": b2b,
              "consts": consts}
    in_maps = []
    for i in range(NCORES):
        s = slice(i * BL, (i + 1) * BL)
        in_maps.append({"hT": np.ascontiguousarray(hT[:, s]),
                        "cT": np.ascontiguousarray(cT[:, s]), **shared})

    nc = _get_nc(dbg)
    res = run_bass_kernel_spmd(nc, in_maps, list(range(NCORES)))
    global LAST_EXEC_NS
    if getattr(res, "exec_time_ns", None):
        LAST_EXEC_NS = res.exec_time_ns
    if dbg:
        return res

    out = np.empty((B, T, 2), np.float32)
    for i in range(NCORES):
        arr = res.results[i]["out"]              # [4, 24576]
        # row = (t%2)*2 + ch ; col = (t//2)*4096 + b
        a4 = arr.reshape(2, 2, T // 2, BL)       # [parity, ch, pair, b]
        out[i * BL:(i + 1) * BL] = a4.transpose(3, 2, 0, 1).reshape(BL, T, 2)
    return out
